# revision 16
# baseline (speedup 1.0000x reference)
"""MoE runtime-experts kernel for 8 Trainium2 NeuronCores.

Problem: y[t] = gelu(x[t] @ W1[e] + b1[e]) @ W2[e] + b2[e], e = indices[t].
T=8192 tokens, D=1024, H=4096, E=8 experts.

Strategy: expert-parallel. Host routes tokens by expert (argsort), core e
gets expert e's weights plus its tokens (transposed, zero-padded to a
common Tp so all 8 cores run one SPMD program). On device each core runs a
dense 2-layer MLP with fp32 PSUM accumulation:

  layer 1: hT[h, t] = gelu(sum_d W1[d, h] * xT[d, t] + b1[h])
           (lhsT = W1 k-tile [128d, 128h], rhs = xT [128d, 384t])
  layer 2: yT[d, t] = sum_h W2[h, d] * hT[h, t] + b2[d]
           (lhsT = W2 h-tile [128h, 128d], rhs = hT [128h, 384t])

Both layers keep the token axis in the free dimension, so no on-device
transpose is needed anywhere — and because tokens are always a free dim,
Tp needs no alignment: every core computes exactly max(counts) token
columns, split into balanced chunks of <=384 (one fp32 PSUM bank each).
Token-chunk DMAs are spread across the sync and gpsimd rings while the
scalar ring streams w1, so the PE starts ~13 us in and stays >=90% busy.
Host un-permutes yT shards into the full [T, 1, D] output.

KERNEL_MODE selects compute dtype: "bf16" (default), "fp8" (both layers
fp8e4m3 + DoubleRow), "fp8l1" (layer 1 fp8, layer 2 bf16).
"""

import math
import os

import numpy as np
import ml_dtypes

T, D, H, E = 8192, 1024, 4096, 8
N_CORES = 8
KB_D = D // 128  # 8  k-tiles of the D contraction
HB = H // 128  # 32 h-tiles
DB = D // 128  # 8  d-tiles
BF16 = ml_dtypes.bfloat16
CS = 384  # token chunk (matmul moving-operand free dim)
SUP = 4 * CS  # tokens resident per pass (SBUF limit)
MM_N = 512  # PSUM bank free size (fp32)

MODE = os.environ.get("KERNEL_MODE", "fp8")

_program_cache: dict[tuple, object] = {}
last_results = None  # BassKernelResults of the most recent kernel() call


def _ed_quant_rows(a: np.ndarray, fp8_np) -> np.ndarray:
    """Error-diffusion quantize each row of [N, D] to fp8, carrying the
    rounding error along D so the row sum is preserved. Plain
    round-to-nearest x-quantization error couples to W1's all-positive
    column means into a token-correlated error that layer 2's all-positive
    W2 amplifies ~10x past the accuracy gate; sum-preserving quantization
    kills that term at zero device cost (measured: rel 0.0225 -> 0.0014)."""
    a = a.astype(np.float32)
    out = np.empty(a.shape, np.float32)
    carry = np.zeros(a.shape[0], np.float32)
    for d in range(a.shape[1]):
        v = a[:, d] + carry
        q = v.astype(fp8_np).astype(np.float32)
        out[:, d] = q
        carry = v - q
    return out


def _chunk_sizes(Tp: int):
    """Balanced split of Tp token columns into chunks of at most CS.
    Sizes are kept even: odd moving-dims measure ~2% slower per column
    on the PE (alignment penalty)."""
    nch = max(1, math.ceil(Tp / CS))
    base, rem = divmod(Tp, nch)
    sizes = [base + (1 if i < rem else 0) for i in range(nch)]
    for i in range(nch - 1):
        if sizes[i] % 2:
            sizes[i] += 1
            sizes[i + 1] -= 1
    return sizes


def _build_program(Tp: int, mode: str):
    import concourse.tile as tile
    from concourse import bacc, mybir

    sizes = _chunk_sizes(Tp)
    nch = len(sizes)
    offs = [sum(sizes[:i]) for i in range(nch)]  # global token offsets

    f32 = mybir.dt.float32
    bf16 = mybir.dt.bfloat16
    fp8 = mybir.dt.float8e4
    l1_dt = fp8 if mode in ("fp8", "fp8l1") else bf16
    l2_dt = fp8 if mode == "fp8" else bf16
    l1_dr = l1_dt == fp8
    l2_dr = l2_dt == fp8
    dr = mybir.MatmulPerfMode.DoubleRow
    gelu = mybir.ActivationFunctionType.Gelu
    ident = mybir.ActivationFunctionType.Identity

    nc = bacc.Bacc(
        "TRN2", target_bir_lowering=False, debug=False, num_devices=N_CORES
    )

    # xq[c] is the SBUF image of token chunk c: [128, KB_D*CS], row-major
    # (kb, t) per partition, so the DMA is fully contiguous
    xq = nc.dram_tensor(
        "xq", [nch, 128, KB_D * CS], l1_dt, kind="ExternalInput"
    ).ap()
    # w1[h] is a [128, KB_D*128] block: col-chunk kb holds W1[kb*128+p, h*128+m]
    w1 = nc.dram_tensor(
        "w1", [HB, 128, KB_D * 128], l1_dt, kind="ExternalInput"
    ).ap()
    # w2[d] is a [128, HB*128] block: col-chunk hb holds W2[hb*128+p, d*128+m]
    w2 = nc.dram_tensor(
        "w2", [DB, 128, HB * 128], l2_dt, kind="ExternalInput"
    ).ap()
    b1 = nc.dram_tensor("b1", [128, HB], f32, kind="ExternalInput").ap()
    b2 = nc.dram_tensor("b2", [128, DB], f32, kind="ExternalInput").ap()
    # bf16 output halves the store traffic; the add runs fp32 on DVE and
    # only the final store rounds (costs ~1e-3 rel err, gate is 2e-2)
    yT = nc.dram_tensor("yT", [D, Tp], bf16, kind="ExternalOutput").ap()

    def mm_group(ps, tsz, nk, lhs_of, rhs_of, use_dr):
        """Accumulate nk k-tiles into psum ps[:, :tsz]; DoubleRow fuses
        pairs of k-tiles per matmul via 3D APs."""
        if use_dr:
            for j in range(0, nk, 2):
                nc.tensor.matmul(
                    ps[:, :tsz],
                    lhs_of(j, 2),
                    rhs_of(j, 2),
                    start=(j == 0),
                    stop=(j == nk - 2),
                    perf_mode=dr,
                )
        else:
            for j in range(nk):
                nc.tensor.matmul(
                    ps[:, :tsz],
                    lhs_of(j, 1),
                    rhs_of(j, 1),
                    start=(j == 0),
                    stop=(j == nk - 1),
                )

    with tile.TileContext(nc) as tc:
        with (
            tc.tile_pool(name="const", bufs=1) as const_pool,
            tc.tile_pool(name="acts", bufs=1) as acts_pool,
            tc.tile_pool(name="xtp", bufs=3) as xt_pool,
            tc.tile_pool(name="w1p", bufs=6) as w1_pool,
            tc.tile_pool(name="w2p", bufs=2) as w2_pool,
            tc.tile_pool(name="outp", bufs=4) as out_pool,
            tc.tile_pool(name="psum", bufs=8, space="PSUM") as psum_pool,
        ):
            b1_sb = const_pool.tile([128, HB], f32)
            b2_sb = const_pool.tile([128, DB], f32)

            for sup0 in range(0, nch, SUP // CS):

                cix = list(range(sup0, min(sup0 + SUP // CS, nch)))
                loffs = [offs[c] - offs[cix[0]] for c in cix]  # ht-local
                sup_len = sum(sizes[c] for c in cix)
                ht_sb = acts_pool.tile([128, HB, sup_len], l2_dt, tag="ht")

                # token chunks: chunk 0 on the scalar ring (gates the first
                # matmul; Scalar is idle until GELUs begin), the rest on the
                # gpsimd ring in parallel; the sync ring carries the w1
                # stream so Scalar stays dedicated to GELU afterwards
                xts = []
                for ci, c in enumerate(cix):
                    xt_c = xt_pool.tile(
                        [128, KB_D, CS], l1_dt, tag=f"xt{ci}", bufs=1
                    )
                    src = xq[c].rearrange("p (k m) -> p k m", k=KB_D)
                    if ci == 0:
                        # split so the first matmul (k-pair 0) only waits
                        # for the first quarter of the chunk
                        nc.scalar.dma_start(xt_c[:, :2], src[:, :2])
                        nc.scalar.dma_start(xt_c[:, 2:], src[:, 2:])
                    else:
                        nc.gpsimd.dma_start(xt_c[:], src)
                    xts.append(xt_c)
                if sup0 == 0:
                    nc.scalar.dma_start(b1_sb[:], b1[:])
                    nc.scalar.dma_start(b2_sb[:], b2[:])

                # ---- layer 1: hT[h, c] ----
                for h in range(HB):
                    w1t = w1_pool.tile([128, KB_D, 128], l1_dt, tag="w1t")
                    # w1 stream rides the sync ring so the Scalar queue
                    # stays dedicated to GELU (it was 75% busy with DMA
                    # descriptors + semaphores stealing from ACT)
                    w1src = w1[h].rearrange("p (k m) -> p k m", k=KB_D)
                    if sup0 == 0 and h == 0:
                        # split so the very first matmul (k-pair 0) only
                        # waits for a quarter of the tile
                        nc.sync.dma_start(w1t[:, :2], w1src[:, :2])
                        nc.sync.dma_start(w1t[:, 2:], w1src[:, 2:])
                    else:
                        nc.sync.dma_start(w1t[:], w1src)
                    for ci, c in enumerate(cix):
                        xt_c = xts[ci]
                        tsz = sizes[c]
                        lo = loffs[ci]
                        ps = psum_pool.tile([128, MM_N], f32, tag="ps")
                        mm_group(
                            ps,
                            tsz,
                            KB_D,
                            lambda j, w: w1t[:, j : j + w, :]
                            if w == 2
                            else w1t[:, j, :],
                            lambda j, w: xt_c[:, j : j + w, :tsz]
                            if w == 2
                            else xt_c[:, j, :tsz],
                            l1_dr,
                        )
                        nc.scalar.activation(
                            ht_sb[:, h, lo : lo + tsz],
                            ps[:, :tsz],
                            gelu,
                            bias=b1_sb[:, h : h + 1],
                        )

                # ---- layer 2: yT[d, c] ----
                for d in range(DB):
                    # w2 on the gpsimd (SWDGE) ring: parallel to the w1
                    # stream on the scalar ring, so d=0 prefetches early
                    w2t = w2_pool.tile([128, HB, 128], l2_dt, tag="w2t")
                    nc.gpsimd.dma_start(
                        w2t[:], w2[d].rearrange("p (k m) -> p k m", k=HB)
                    )
                    for ci, c in enumerate(cix):
                        tsz = sizes[c]
                        lo = loffs[ci]
                        go = offs[c]
                        ps = psum_pool.tile([128, MM_N], f32, tag="ps")
                        mm_group(
                            ps,
                            tsz,
                            HB,
                            lambda j, w: w2t[:, j : j + w, :]
                            if w == 2
                            else w2t[:, j, :],
                            lambda j, w: ht_sb[:, j : j + w, lo : lo + tsz]
                            if w == 2
                            else ht_sb[:, j, lo : lo + tsz],
                            l2_dr,
                        )
                        ot = out_pool.tile([128, MM_N], bf16, tag="ot")
                        # final d-tile: split each chunk's add+store into
                        # small pieces on alternating rings so the exposed
                        # tail after the last matmul shrinks
                        if d == DB - 1:
                            third = tsz // 3
                            pieces = [
                                (0, third),
                                (third, third),
                                (2 * third, tsz - 2 * third),
                            ]
                        else:
                            pieces = [(0, tsz)]
                        rings = [nc.sync, nc.scalar, nc.gpsimd]
                        for pi, (p0, psz) in enumerate(pieces):
                            # bias add on DVE, not Scalar ACT: keeps the
                            # Scalar queue free and overlaps layer-2 tail
                            nc.vector.tensor_scalar_add(
                                ot[:, p0 : p0 + psz],
                                ps[:, p0 : p0 + psz],
                                b2_sb[:, d : d + 1],
                            )
                            st_eng = rings[(d * len(cix) + ci + pi) % 3]
                            st_eng.dma_start(
                                yT[
                                    d * 128 : (d + 1) * 128,
                                    go + p0 : go + p0 + psz,
                                ],
                                ot[:, p0 : p0 + psz],
                            )

    nc.compile()
    return nc


def kernel(x, indices_s, weight1, weight2, bias1, bias2):
    from concourse import mybir
    from concourse.bass_utils import run_bass_kernel_spmd

    x = np.asarray(x, dtype=np.float32)
    if MODE in ("fp8", "fp8l1"):
        x = _ed_quant_rows(x, mybir.dt.np(mybir.dt.float8e4))
    idx = np.asarray(indices_s).astype(np.int64).ravel()
    w1_full = np.asarray(weight1, dtype=np.float32)
    w2_full = np.asarray(weight2, dtype=np.float32)
    b1_full = np.asarray(bias1, dtype=np.float32)
    b2_full = np.asarray(bias2, dtype=np.float32)

    order = np.argsort(idx, kind="stable")
    counts = np.bincount(idx, minlength=E)
    starts = np.concatenate([[0], np.cumsum(counts)])
    # tokens live in the free dim everywhere, so no alignment is needed:
    # every core computes exactly max(counts) token columns
    Tp = max(128, int(counts.max()))
    sizes = _chunk_sizes(Tp)
    nch = len(sizes)
    offs = np.concatenate([[0], np.cumsum(sizes)])

    mode = MODE
    key = (Tp, mode)
    nc = _program_cache.get(key)
    if nc is None:
        nc = _build_program(Tp, mode)
        _program_cache[key] = nc

    fp8_np = mybir.dt.np(mybir.dt.float8e4)
    l1_np = fp8_np if mode in ("fp8", "fp8l1") else BF16
    l2_np = fp8_np if mode == "fp8" else BF16

    in_maps = []
    for e in range(E):
        toks = order[starts[e] : starts[e + 1]]
        # slot-aligned image: chunk c's tokens at columns [c*CS, c*CS+sizes[c])
        xTs = np.zeros((D, nch * CS), dtype=np.float32)
        for c in range(nch):
            lo, hi = offs[c], min(offs[c + 1], counts[e])
            if hi > lo:
                xTs[:, c * CS : c * CS + (hi - lo)] = x[toks[lo:hi]].T
        # [D, nch*CS] -> [nch, 128, KB_D*CS] chunk-major SBUF image
        xq = (
            np.ascontiguousarray(
                xTs.reshape(KB_D, 128, nch, CS).transpose(2, 1, 0, 3)
            )
            .reshape(nch, 128, KB_D * CS)
            .astype(l1_np)
        )
        w1r = (
            np.ascontiguousarray(
                w1_full[e].reshape(KB_D, 128, HB, 128).transpose(2, 1, 0, 3)
            )
            .reshape(HB, 128, KB_D * 128)
            .astype(l1_np)
        )
        w2r = (
            np.ascontiguousarray(
                w2_full[e].reshape(HB, 128, DB, 128).transpose(2, 1, 0, 3)
            )
            .reshape(DB, 128, HB * 128)
            .astype(l2_np)
        )
        b1d = np.ascontiguousarray(b1_full[e].reshape(HB, 128).T)
        b2d = np.ascontiguousarray(b2_full[e].reshape(DB, 128).T)
        in_maps.append({"xq": xq, "w1": w1r, "w2": w2r, "b1": b1d, "b2": b2d})

    res = run_bass_kernel_spmd(
        nc,
        in_maps,
        list(range(N_CORES)),
        trace=os.environ.get("BASS_TRACE") == "1",
    )
    global last_results
    last_results = res

    out = np.empty((T, D), dtype=np.float32)
    for e in range(E):
        toks = order[starts[e] : starts[e + 1]]
        out[toks] = res.results[e]["yT"][:, : counts[e]].T.astype(np.float32)
    if res.exec_time_ns is not None:
        print(f"HW exec time: {res.exec_time_ns} ns")
    return out[:, None, :]



# revision 20
# speedup vs baseline: 1.0049x; 1.0049x over previous
"""MoE runtime-experts kernel for 8 Trainium2 NeuronCores.

Problem: y[t] = gelu(x[t] @ W1[e] + b1[e]) @ W2[e] + b2[e], e = indices[t].
T=8192 tokens, D=1024, H=4096, E=8 experts.

Strategy: expert-parallel. Host routes tokens by expert (argsort), core e
gets expert e's weights plus its tokens (transposed, zero-padded to a
common Tp so all 8 cores run one SPMD program). On device each core runs a
dense 2-layer MLP with fp32 PSUM accumulation:

  layer 1: hT[h, t] = gelu(sum_d W1[d, h] * xT[d, t] + b1[h])
           (lhsT = W1 k-tile [128d, 128h], rhs = xT [128d, 384t])
  layer 2: yT[d, t] = sum_h W2[h, d] * hT[h, t] + b2[d]
           (lhsT = W2 h-tile [128h, 128d], rhs = hT [128h, 384t])

Both layers keep the token axis in the free dimension, so no on-device
transpose is needed anywhere — and because tokens are always a free dim,
Tp needs no alignment: every core computes exactly max(counts) token
columns, split into balanced chunks of <=384 (one fp32 PSUM bank each).
Token-chunk DMAs are spread across the sync and gpsimd rings while the
scalar ring streams w1, so the PE starts ~13 us in and stays >=90% busy.
Host un-permutes yT shards into the full [T, 1, D] output.

KERNEL_MODE selects compute dtype: "bf16" (default), "fp8" (both layers
fp8e4m3 + DoubleRow), "fp8l1" (layer 1 fp8, layer 2 bf16).
"""

import math
import os

import numpy as np
import ml_dtypes

T, D, H, E = 8192, 1024, 4096, 8
N_CORES = 8
KB_D = D // 128  # 8  k-tiles of the D contraction
HB = H // 128  # 32 h-tiles
DB = D // 128  # 8  d-tiles
BF16 = ml_dtypes.bfloat16
CS = 384  # token chunk (matmul moving-operand free dim)
SUP = 4 * CS  # tokens resident per pass (SBUF limit)
MM_N = 512  # PSUM bank free size (fp32)

MODE = os.environ.get("KERNEL_MODE", "fp8")

_program_cache: dict[tuple, object] = {}
last_results = None  # BassKernelResults of the most recent kernel() call


def _ed_quant_rows(a: np.ndarray, fp8_np) -> np.ndarray:
    """Error-diffusion quantize each row of [N, D] to fp8, carrying the
    rounding error along D so the row sum is preserved. Plain
    round-to-nearest x-quantization error couples to W1's all-positive
    column means into a token-correlated error that layer 2's all-positive
    W2 amplifies ~10x past the accuracy gate; sum-preserving quantization
    kills that term at zero device cost (measured: rel 0.0225 -> 0.0014)."""
    a = a.astype(np.float32)
    out = np.empty(a.shape, np.float32)
    carry = np.zeros(a.shape[0], np.float32)
    for d in range(a.shape[1]):
        v = a[:, d] + carry
        q = v.astype(fp8_np).astype(np.float32)
        out[:, d] = q
        carry = v - q
    return out


def _chunk_sizes(Tp: int):
    """Balanced split of Tp token columns into chunks of at most CS.
    Sizes are kept even: odd moving-dims measure ~2% slower per column
    on the PE (alignment penalty)."""
    nch = max(1, math.ceil(Tp / CS))
    base, rem = divmod(Tp, nch)
    sizes = [base + (1 if i < rem else 0) for i in range(nch)]
    for i in range(nch - 1):
        if sizes[i] % 2:
            sizes[i] += 1
            sizes[i + 1] -= 1
    return sizes


def _build_program(Tp: int, mode: str):
    import concourse.tile as tile
    from concourse import bacc, mybir

    sizes = _chunk_sizes(Tp)
    nch = len(sizes)
    offs = [sum(sizes[:i]) for i in range(nch)]  # global token offsets

    f32 = mybir.dt.float32
    bf16 = mybir.dt.bfloat16
    fp8 = mybir.dt.float8e4
    l1_dt = fp8 if mode in ("fp8", "fp8l1") else bf16
    l2_dt = fp8 if mode == "fp8" else bf16
    l1_dr = l1_dt == fp8
    l2_dr = l2_dt == fp8
    dr = mybir.MatmulPerfMode.DoubleRow
    gelu = mybir.ActivationFunctionType.Gelu
    ident = mybir.ActivationFunctionType.Identity

    nc = bacc.Bacc(
        "TRN2", target_bir_lowering=False, debug=False, num_devices=N_CORES
    )

    # xq[c] is the SBUF image of token chunk c: [128, KB_D*CS], row-major
    # (kb, t) per partition, so the DMA is fully contiguous
    xq = nc.dram_tensor(
        "xq", [nch, 128, KB_D * CS], l1_dt, kind="ExternalInput"
    ).ap()
    # w1[h] is a [128, KB_D*128] block: col-chunk kb holds W1[kb*128+p, h*128+m]
    w1 = nc.dram_tensor(
        "w1", [HB, 128, KB_D * 128], l1_dt, kind="ExternalInput"
    ).ap()
    # w2[d] is a [128, HB*128] block: col-chunk hb holds W2[hb*128+p, d*128+m]
    w2 = nc.dram_tensor(
        "w2", [DB, 128, HB * 128], l2_dt, kind="ExternalInput"
    ).ap()
    b1 = nc.dram_tensor("b1", [128, HB], f32, kind="ExternalInput").ap()
    b2 = nc.dram_tensor("b2", [128, DB], f32, kind="ExternalInput").ap()
    # bf16 output halves the store traffic; the add runs fp32 on DVE and
    # only the final store rounds (costs ~1e-3 rel err, gate is 2e-2)
    yT = nc.dram_tensor("yT", [D, Tp], bf16, kind="ExternalOutput").ap()

    def mm_group(ps, tsz, nk, lhs_of, rhs_of, use_dr):
        """Accumulate nk k-tiles into psum ps[:, :tsz]; DoubleRow fuses
        pairs of k-tiles per matmul via 3D APs."""
        if use_dr:
            for j in range(0, nk, 2):
                nc.tensor.matmul(
                    ps[:, :tsz],
                    lhs_of(j, 2),
                    rhs_of(j, 2),
                    start=(j == 0),
                    stop=(j == nk - 2),
                    perf_mode=dr,
                )
        else:
            for j in range(nk):
                nc.tensor.matmul(
                    ps[:, :tsz],
                    lhs_of(j, 1),
                    rhs_of(j, 1),
                    start=(j == 0),
                    stop=(j == nk - 1),
                )

    with tile.TileContext(nc) as tc:
        with (
            tc.tile_pool(name="const", bufs=1) as const_pool,
            tc.tile_pool(name="acts", bufs=1) as acts_pool,
            tc.tile_pool(name="xtp", bufs=3) as xt_pool,
            tc.tile_pool(name="w1p", bufs=6) as w1_pool,
            tc.tile_pool(name="w2p", bufs=2) as w2_pool,
            tc.tile_pool(name="outp", bufs=4) as out_pool,
            tc.tile_pool(name="psum", bufs=8, space="PSUM") as psum_pool,
        ):
            b1_sb = const_pool.tile([128, HB], f32)
            b2_sb = const_pool.tile([128, DB], f32)

            for sup0 in range(0, nch, SUP // CS):

                cix = list(range(sup0, min(sup0 + SUP // CS, nch)))
                loffs = [offs[c] - offs[cix[0]] for c in cix]  # ht-local
                sup_len = sum(sizes[c] for c in cix)
                ht_sb = acts_pool.tile([128, HB, sup_len], l2_dt, tag="ht")

                # token chunks: chunk 0 on the scalar ring (gates the first
                # matmul; Scalar is idle until GELUs begin), the rest on the
                # gpsimd ring in parallel; the sync ring carries the w1
                # stream so Scalar stays dedicated to GELU afterwards
                # the very first matmul needs only k-pair 0 of w1[0] and of
                # chunk 0: issue those two small transfers first, on
                # different rings, so they land in parallel ~2us in
                w1t0 = None
                if sup0 == 0:
                    w1t0 = w1_pool.tile([128, KB_D, 128], l1_dt, tag="w1t")
                    w1src0 = w1[0].rearrange("p (k m) -> p k m", k=KB_D)
                    nc.gpsimd.dma_start(w1t0[:, :2], w1src0[:, :2])

                xts = []
                for ci, c in enumerate(cix):
                    xt_c = xt_pool.tile(
                        [128, KB_D, CS], l1_dt, tag=f"xt{ci}", bufs=1
                    )
                    src = xq[c].rearrange("p (k m) -> p k m", k=KB_D)
                    if ci == 0:
                        # split so the first matmul (k-pair 0) only waits
                        # for the first quarter of the chunk
                        nc.scalar.dma_start(xt_c[:, :2], src[:, :2])
                        nc.scalar.dma_start(xt_c[:, 2:], src[:, 2:])
                    else:
                        # halves unblock h=0's chunk groups sooner
                        nc.gpsimd.dma_start(xt_c[:, :4], src[:, :4])
                        nc.gpsimd.dma_start(xt_c[:, 4:], src[:, 4:])
                    xts.append(xt_c)
                if sup0 == 0:
                    nc.scalar.dma_start(b1_sb[:], b1[:])
                    nc.scalar.dma_start(b2_sb[:], b2[:])

                # ---- layer 1: hT[h, c] ----
                for h in range(HB):
                    # w1 stream rides the sync ring so the Scalar queue
                    # stays dedicated to GELU (it was 75% busy with DMA
                    # descriptors + semaphores stealing from ACT)
                    w1src = w1[h].rearrange("p (k m) -> p k m", k=KB_D)
                    if w1t0 is not None and h == 0:
                        w1t = w1t0
                        nc.sync.dma_start(w1t[:, 2:], w1src[:, 2:])
                    else:
                        w1t = w1_pool.tile([128, KB_D, 128], l1_dt, tag="w1t")
                        nc.sync.dma_start(w1t[:], w1src)
                    for ci, c in enumerate(cix):
                        xt_c = xts[ci]
                        tsz = sizes[c]
                        lo = loffs[ci]
                        ps = psum_pool.tile([128, MM_N], f32, tag="ps")
                        mm_group(
                            ps,
                            tsz,
                            KB_D,
                            lambda j, w: w1t[:, j : j + w, :]
                            if w == 2
                            else w1t[:, j, :],
                            lambda j, w: xt_c[:, j : j + w, :tsz]
                            if w == 2
                            else xt_c[:, j, :tsz],
                            l1_dr,
                        )
                        nc.scalar.activation(
                            ht_sb[:, h, lo : lo + tsz],
                            ps[:, :tsz],
                            gelu,
                            bias=b1_sb[:, h : h + 1],
                        )

                # ---- layer 2: yT[d, c] ----
                for d in range(DB):
                    # w2 on the gpsimd (SWDGE) ring: parallel to the w1
                    # stream on the scalar ring, so d=0 prefetches early
                    w2t = w2_pool.tile([128, HB, 128], l2_dt, tag="w2t")
                    nc.gpsimd.dma_start(
                        w2t[:], w2[d].rearrange("p (k m) -> p k m", k=HB)
                    )
                    for ci, c in enumerate(cix):
                        tsz = sizes[c]
                        lo = loffs[ci]
                        go = offs[c]
                        # the very last matmul group is split column-wise
                        # so its first half's add+store overlap the second
                        # half's matmuls, shrinking the exposed tail
                        last = d == DB - 1 and c == cix[-1]
                        col_ranges = (
                            [(0, tsz // 2), (tsz // 2, tsz)]
                            if last
                            else [(0, tsz)]
                        )
                        rings = [nc.sync, nc.scalar, nc.gpsimd]
                        for r0, r1 in col_ranges:
                            rn = r1 - r0
                            ps = psum_pool.tile([128, MM_N], f32, tag="ps")
                            mm_group(
                                ps,
                                rn,
                                HB,
                                lambda j, w: w2t[:, j : j + w, :]
                                if w == 2
                                else w2t[:, j, :],
                                lambda j, w: ht_sb[
                                    :, j : j + w, lo + r0 : lo + r1
                                ]
                                if w == 2
                                else ht_sb[:, j, lo + r0 : lo + r1],
                                l2_dr,
                            )
                            ot = out_pool.tile([128, MM_N], bf16, tag="ot")
                            # final d-tile: split each range's add+store
                            # into pieces on alternating rings so no ring
                            # backlog extends the tail
                            if d == DB - 1 and rn > 160:
                                half = rn // 2
                                pieces = [(0, half), (half, rn - half)]
                            else:
                                pieces = [(0, rn)]
                            for pi, (p0, psz) in enumerate(pieces):
                                # bias add on DVE, not Scalar ACT: keeps
                                # the Scalar queue free and overlaps the
                                # layer-2 tail
                                nc.vector.tensor_scalar_add(
                                    ot[:, p0 : p0 + psz],
                                    ps[:, p0 : p0 + psz],
                                    b2_sb[:, d : d + 1],
                                )
                                st_eng = rings[
                                    (d * len(cix) + ci + pi + r0) % 3
                                ]
                                st_eng.dma_start(
                                    yT[
                                        d * 128 : (d + 1) * 128,
                                        go + r0 + p0 : go + r0 + p0 + psz,
                                    ],
                                    ot[:, p0 : p0 + psz],
                                )

    nc.compile()
    return nc


def kernel(x, indices_s, weight1, weight2, bias1, bias2):
    from concourse import mybir
    from concourse.bass_utils import run_bass_kernel_spmd

    x = np.asarray(x, dtype=np.float32)
    if MODE in ("fp8", "fp8l1"):
        x = _ed_quant_rows(x, mybir.dt.np(mybir.dt.float8e4))
    idx = np.asarray(indices_s).astype(np.int64).ravel()
    w1_full = np.asarray(weight1, dtype=np.float32)
    w2_full = np.asarray(weight2, dtype=np.float32)
    b1_full = np.asarray(bias1, dtype=np.float32)
    b2_full = np.asarray(bias2, dtype=np.float32)

    order = np.argsort(idx, kind="stable")
    counts = np.bincount(idx, minlength=E)
    starts = np.concatenate([[0], np.cumsum(counts)])
    # tokens live in the free dim everywhere, so no alignment is needed:
    # every core computes exactly max(counts) token columns
    Tp = max(128, int(counts.max()))
    sizes = _chunk_sizes(Tp)
    nch = len(sizes)
    offs = np.concatenate([[0], np.cumsum(sizes)])

    mode = MODE
    key = (Tp, mode)
    nc = _program_cache.get(key)
    if nc is None:
        nc = _build_program(Tp, mode)
        _program_cache[key] = nc

    fp8_np = mybir.dt.np(mybir.dt.float8e4)
    l1_np = fp8_np if mode in ("fp8", "fp8l1") else BF16
    l2_np = fp8_np if mode == "fp8" else BF16

    in_maps = []
    for e in range(E):
        toks = order[starts[e] : starts[e + 1]]
        # slot-aligned image: chunk c's tokens at columns [c*CS, c*CS+sizes[c])
        xTs = np.zeros((D, nch * CS), dtype=np.float32)
        for c in range(nch):
            lo, hi = offs[c], min(offs[c + 1], counts[e])
            if hi > lo:
                xTs[:, c * CS : c * CS + (hi - lo)] = x[toks[lo:hi]].T
        # [D, nch*CS] -> [nch, 128, KB_D*CS] chunk-major SBUF image
        xq = (
            np.ascontiguousarray(
                xTs.reshape(KB_D, 128, nch, CS).transpose(2, 1, 0, 3)
            )
            .reshape(nch, 128, KB_D * CS)
            .astype(l1_np)
        )
        w1r = (
            np.ascontiguousarray(
                w1_full[e].reshape(KB_D, 128, HB, 128).transpose(2, 1, 0, 3)
            )
            .reshape(HB, 128, KB_D * 128)
            .astype(l1_np)
        )
        w2r = (
            np.ascontiguousarray(
                w2_full[e].reshape(HB, 128, DB, 128).transpose(2, 1, 0, 3)
            )
            .reshape(DB, 128, HB * 128)
            .astype(l2_np)
        )
        b1d = np.ascontiguousarray(b1_full[e].reshape(HB, 128).T)
        b2d = np.ascontiguousarray(b2_full[e].reshape(DB, 128).T)
        in_maps.append({"xq": xq, "w1": w1r, "w2": w2r, "b1": b1d, "b2": b2d})

    res = run_bass_kernel_spmd(
        nc,
        in_maps,
        list(range(N_CORES)),
        trace=os.environ.get("BASS_TRACE") == "1",
    )
    global last_results
    last_results = res

    out = np.empty((T, D), dtype=np.float32)
    for e in range(E):
        toks = order[starts[e] : starts[e + 1]]
        out[toks] = res.results[e]["yT"][:, : counts[e]].T.astype(np.float32)
    if res.exec_time_ns is not None:
        print(f"HW exec time: {res.exec_time_ns} ns")
    return out[:, None, :]

